# revision 27
# baseline (speedup 1.0000x reference)
"""Grouped multi-query attention on 8 trn2 NeuronCores.

Sharding: kv-head groups x batch. Core c handles batch b = c // 4 and
kv-head pair p = c % 4 (kv heads 2p, 2p+1 and their 8 query heads), over
the full 2048-token sequence of its batch. The output projection is
row-parallel over the core's 512 o-dims, producing a full-shape partial
that the host sums (no device collectives).

Everything on the PE runs at 1 cycle/row: projections and attention in
bf16, the reciprocal-broadcast in fp32r. Scores are computed transposed
[m, n] per head with both operands at partition base 64*h so no data
shuffles are needed. Softmax denominators come from an appended
ones-column in the attn@v matmul; exp runs on the scalar engine
(bf16 out, scale=1/8 folded in) with 1/4 of score chunks handled by a
one-op Schraudolph exp on DVE so the scalar engine stays off the
critical path. Input DMAs are issued from both HW-DGE rings (SP + Act).

Per-core PE work: q-proj 131k + k/v-proj 66k + scores 262k + attn@v 262k
+ o-proj 131k + misc 18k ~= 870k cycles ~= 363us at 2.4 GHz.
"""
import sys
sys.path.insert(0, '/opt/trn_rl_repo')

import numpy as np
import ml_dtypes

D = 2048          # d_model
N = 2048          # sequence length
DK = 64           # head dim
NB = 512          # n-block for attention / psum tiles
_NC_CACHE = {}

BF = ml_dtypes.bfloat16

# Schraudolph exp on DVE for scores chunks with mc % SCHR_EVERY ==
# SCHR_EVERY - 1 (0 disables). bits16 = trunc(s_raw * 184.662/8 + 16245.81)
# viewed as bf16: a one-op DVE "exp" that offloads the Act engine; rel err
# ~2.9% max on those tiles' weights, ~1e-2 end-to-end at 1/4 of tiles.
SCHR_EVERY = 4
S1 = 184.662 / 8.0
S2 = 16256.0 - 10.69 + 0.5


def _build_nc():
    import concourse.bacc as bacc
    import concourse.mybir as mybir
    from concourse import tile

    F32 = mybir.dt.float32
    F32R = mybir.dt.float32r
    BF16 = mybir.dt.bfloat16

    nc = bacc.Bacc("TRN2", target_bir_lowering=False, debug=False)

    qt = nc.dram_tensor("qt", [128, 16, N], BF16, kind="ExternalInput").ap()
    kt = nc.dram_tensor("kt", [128, 16, N], BF16, kind="ExternalInput").ap()
    vt = nc.dram_tensor("vt", [128, 16, N], BF16, kind="ExternalInput").ap()
    wqp = nc.dram_tensor("wqp", [128, 16, 512], BF16, kind="ExternalInput").ap()
    wkp = nc.dram_tensor("wkp", [128, 16, 128], BF16, kind="ExternalInput").ap()
    wvp = nc.dram_tensor("wvp", [128, 16, 128], BF16, kind="ExternalInput").ap()
    wop = nc.dram_tensor("wop", [128, 4, D], BF16, kind="ExternalInput").ap()
    eye = nc.dram_tensor("eye", [128, 128], F32, kind="ExternalInput").ap()
    outt = nc.dram_tensor("outt", [D, N], BF16, kind="ExternalOutput").ap()

    with tile.TileContext(nc) as tc:
        with (
            tc.tile_pool(name="persist", bufs=1) as persist,
            tc.tile_pool(name="small", bufs=4) as small,
        ):
            q_sb = persist.tile([128, 4, N], BF16, tag="q_sb")
            kt_sb = persist.tile([128, N], BF16, tag="kt_sb")
            v_nat = persist.tile([128, 16, 2, 65], BF16, tag="v_nat")
            o_sb = persist.tile([128, 4, N], BF16, tag="o_sb")
            wo_sb = persist.tile([128, 4, D], BF16, tag="wo_sb")
            wq_sb = persist.tile([128, 16, 512], BF16, tag="wq_sb")
            wk_sb = persist.tile([128, 16, 128], BF16, tag="wk_sb")
            wv_sb = persist.tile([128, 16, 128], BF16, tag="wv_sb")
            eye_sb = persist.tile([128, 128], F32, tag="eye_sb")
            ones1 = persist.tile([65, 64], F32R, tag="ones1")
            zbias = persist.tile([128, 1], F32, tag="zbias")

            nc.scalar.dma_start(wk_sb[:], wkp[:])
            nc.sync.dma_start(eye_sb[:], eye[:])
            onesf = small.tile([65, 64], F32, tag="onesf")
            nc.vector.memset(onesf[:], 1.0)
            nc.vector.tensor_copy(ones1[:], onesf[:])
            nc.vector.memset(zbias[:], 0.0)
            nc.vector.memset(v_nat[:, :, :, 64:65], 1.0)

            # ---------------- projections ----------------
            with (
                tc.tile_pool(name="xt", bufs=6) as xtp,
                tc.tile_pool(name="ppsum", bufs=6, space="PSUM") as pp,
                tc.tile_pool(name="trpsum", bufs=2, space="PSUM") as trpp,
                tc.tile_pool(name="vtr", bufs=2) as vtrp,
            ):
                # k-projection: kt_sb[64h+d, m] (order matches scores lhsT)
                kq = [xtp.tile([128, 4, N], BF16, tag="xt", name=f"kq{i}")
                      for i in range(4)]
                for i in range(4):
                    eng = nc.sync if i % 2 == 0 else nc.scalar
                    eng.dma_start(kq[i][:], kt[:, 4 * i:4 * i + 4, :])
                nc.sync.dma_start(wv_sb[:], wvp[:])
                ps = [pp.tile([128, NB], F32, tag="pp", name=f"kp{i}")
                      for i in range(4)]
                for dc in range(16):
                    for nb in range(4):
                        nc.tensor.matmul(
                            ps[nb][:], wk_sb[:, dc, :],
                            kq[dc // 4][:, dc % 4, nb * NB:(nb + 1) * NB],
                            start=(dc == 0), stop=(dc == 15))
                for nb in range(4):
                    nc.vector.tensor_copy(kt_sb[:, nb * NB:(nb + 1) * NB],
                                          ps[nb][:])

                # v-projection + PE-transpose into v_nat
                vq = [xtp.tile([128, 4, N], BF16, tag="xt", name=f"vq{i}")
                      for i in range(4)]
                for i in range(4):
                    eng = nc.sync if i % 2 == 1 else nc.scalar
                    eng.dma_start(vq[i][:], vt[:, 4 * i:4 * i + 4, :])
                nc.scalar.dma_start(wq_sb[:], wqp[:])
                ps = [pp.tile([128, NB], F32, tag="pp", name=f"vp{i}")
                      for i in range(4)]
                for dc in range(16):
                    for nb in range(4):
                        nc.tensor.matmul(
                            ps[nb][:], wv_sb[:, dc, :],
                            vq[dc // 4][:, dc % 4, nb * NB:(nb + 1) * NB],
                            start=(dc == 0), stop=(dc == 15))
                for nb in range(4):
                    vtmp = vtrp.tile([128, NB], F32, tag="vtmp")
                    nc.vector.tensor_copy(vtmp[:], ps[nb][:])
                    trp = trpp.tile([128, NB], F32, tag="trp", name=f"vt{nb}")
                    for q in range(4):
                        nc.tensor.transpose(trp[:, q * 128:(q + 1) * 128],
                                            vtmp[:, q * 128:(q + 1) * 128],
                                            eye_sb[:])
                    for q in range(4):
                        mc = nb * 4 + q
                        for h in range(2):
                            nc.vector.tensor_copy(
                                v_nat[:, mc, h, 0:64],
                                trp[:, q * 128 + h * 64:q * 128 + h * 64 + 64])

                # q-projection: q_sb[64h+d, g, n]
                qq = [xtp.tile([128, 4, N], BF16, tag="xt", name=f"qq{i}")
                      for i in range(4)]
                for i in range(4):
                    eng = nc.sync if i % 2 == 0 else nc.scalar
                    eng.dma_start(qq[i][:], qt[:, 4 * i:4 * i + 4, :])
                nc.scalar.dma_start(wo_sb[:], wop[:])
                for jc in range(4):
                    ps = [pp.tile([128, NB], F32, tag="pp", name=f"qp{jc}_{i}")
                          for i in range(4)]
                    for dc in range(16):
                        for nb in range(4):
                            nc.tensor.matmul(
                                ps[nb][:], wq_sb[:, dc, jc * 128:(jc + 1) * 128],
                                qq[dc // 4][:, dc % 4, nb * NB:(nb + 1) * NB],
                                start=(dc == 0), stop=(dc == 15))
                    for nb in range(4):
                        nc.vector.tensor_copy(
                            q_sb[:, jc, nb * NB:(nb + 1) * NB], ps[nb][:])

            # ---------------- attention ----------------
            with (
                tc.tile_pool(name="scp", bufs=2, space="PSUM") as scp,
                tc.tile_pool(name="pop", bufs=2, space="PSUM") as pop,
                tc.tile_pool(name="pbp", bufs=2, space="PSUM") as pbp,
                tc.tile_pool(name="expp", bufs=2) as expp,
                tc.tile_pool(name="att_sm", bufs=4) as asm,
                tc.tile_pool(name="oshift", bufs=2) as osh,
            ):
                for h in range(2):
                    for g in range(4):
                        hg = 4 * h + g
                        q_ap = q_sb[64 * h:64 * h + 64, g, :]
                        tmp_odd = None
                        if hg % 2 == 1:
                            tmp_odd = osh.tile([64, N], BF16, tag="tmp_odd")
                        for nb2 in range(2):
                            expT = expp.tile([128, 16, 1024], BF16, tag="expT")
                            for mc in range(16):
                                sc = scp.tile([128, 1024], F32, tag="sc")
                                for i in range(2):
                                    nc.tensor.matmul(
                                        sc[:, i * NB:(i + 1) * NB],
                                        kt_sb[64 * h:64 * h + 64,
                                              mc * 128:(mc + 1) * 128],
                                        q_ap[:, nb2 * 1024 + i * NB:
                                             nb2 * 1024 + (i + 1) * NB],
                                        start=True, stop=True)
                                if SCHR_EVERY and mc % SCHR_EVERY == SCHR_EVERY - 1:
                                    nc.vector.tensor_scalar(
                                        expT[:, mc, :].bitcast(mybir.dt.int16),
                                        sc[:], S1, S2,
                                        mybir.AluOpType.mult,
                                        mybir.AluOpType.add)
                                else:
                                    nc.scalar.activation(
                                        expT[:, mc, :], sc[:],
                                        mybir.ActivationFunctionType.Exp,
                                        bias=zbias[:], scale=0.125)
                            for nbi in range(2):
                                nb = nb2 * 2 + nbi
                                po = pop.tile([65, NB], F32, tag="po")
                                for mc in range(16):
                                    nc.tensor.matmul(
                                        po[:], v_nat[:, mc, h, :],
                                        expT[:, mc, nbi * NB:(nbi + 1) * NB],
                                        start=(mc == 0), stop=(mc == 15))
                                rcp = asm.tile([65, NB], F32, tag="rcp")
                                nc.vector.reciprocal(rcp[64:65, :], po[64:65, :])
                                rcr = asm.tile([65, NB], F32R, tag="rcr")
                                nc.gpsimd.tensor_copy(rcr[64:65, :], rcp[64:65, :])
                                pb = pbp.tile([64, NB], F32, tag="pb")
                                nc.tensor.matmul(pb[:], ones1[64:65, :],
                                                 rcr[64:65, :],
                                                 start=True, stop=True)
                                bc = asm.tile([64, NB], F32, tag="bc")
                                nc.vector.tensor_copy(bc[:], pb[:])
                                if hg % 2 == 0:
                                    nc.vector.tensor_tensor(
                                        o_sb[0:64, hg // 2,
                                             nb * NB:(nb + 1) * NB],
                                        po[0:64, :], bc[:],
                                        mybir.AluOpType.mult)
                                else:
                                    nc.vector.tensor_tensor(
                                        tmp_odd[:, nb * NB:(nb + 1) * NB],
                                        po[0:64, :], bc[:],
                                        mybir.AluOpType.mult)
                        if hg % 2 == 1:
                            nc.sync.dma_start(o_sb[64:128, hg // 2, :],
                                              tmp_odd[:])

            # ---------------- output projection ----------------
            with (
                tc.tile_pool(name="opsum", bufs=8, space="PSUM") as op,
                tc.tile_pool(name="ostg", bufs=2) as ostg,
            ):
                for jc in range(16):
                    ps = [op.tile([128, NB], F32, tag="op", name=f"o{i}")
                          for i in range(4)]
                    for odc in range(4):
                        for nb in range(4):
                            nc.tensor.matmul(
                                ps[nb][:],
                                wo_sb[:, odc, jc * 128:(jc + 1) * 128],
                                o_sb[:, odc, nb * NB:(nb + 1) * NB],
                                start=(odc == 0), stop=(odc == 3))
                    ost = ostg.tile([128, N], BF16, tag="ost")
                    for nb in range(4):
                        if nb % 2 == 0:
                            nc.scalar.activation(
                                ost[:, nb * NB:(nb + 1) * NB], ps[nb][:],
                                mybir.ActivationFunctionType.Copy,
                                bias=0.0)
                        else:
                            nc.vector.tensor_copy(
                                ost[:, nb * NB:(nb + 1) * NB], ps[nb][:])
                    nc.sync.dma_start(outt[jc * 128:(jc + 1) * 128, :], ost[:])
    nc.compile()
    return nc


def get_nc():
    if "nc" not in _NC_CACHE:
        _NC_CACHE["nc"] = _build_nc()
    return _NC_CACHE["nc"]


def _pack_xt(X):
    """[n, d] fp32 -> [128, 16, n] bf16 with d = dc*128 + part."""
    return np.ascontiguousarray(
        X.T.reshape(16, 128, N).transpose(1, 0, 2)).astype(BF)


def make_in_maps(Q, K, V, w_q, w_k, w_v, w_o):
    eye = np.eye(128, dtype=np.float32)
    qts = [_pack_xt(np.asarray(Q[b], dtype=np.float32)) for b in range(2)]
    kts = [_pack_xt(np.asarray(K[b], dtype=np.float32)) for b in range(2)]
    vts = [_pack_xt(np.asarray(V[b], dtype=np.float32)) for b in range(2)]

    w_q = np.asarray(w_q, dtype=np.float32)
    w_k = np.asarray(w_k, dtype=np.float32)
    w_v = np.asarray(w_v, dtype=np.float32)
    w_o = np.asarray(w_o, dtype=np.float32)

    in_maps = []
    for c in range(8):
        b, p = c // 4, c % 4
        # w_q rows for heads (2p+h, g), j-order = g*128 + h*64 + d
        wq_sel = w_q.reshape(8, 4, 64, D)[2 * p:2 * p + 2]      # [h, g, d, :]
        jmat = wq_sel.transpose(1, 0, 2, 3).reshape(512, D)     # j=(g,h,d)
        wqp = np.ascontiguousarray(
            jmat.T.reshape(16, 128, 512).transpose(1, 0, 2)).astype(BF)
        wk_sel = w_k.reshape(8, 64, D)[2 * p:2 * p + 2].reshape(128, D)
        wkp = np.ascontiguousarray(
            wk_sel.T.reshape(16, 128, 128).transpose(1, 0, 2)).astype(BF)
        wv_sel = w_v.reshape(8, 64, D)[2 * p:2 * p + 2].reshape(128, D)
        wvp = np.ascontiguousarray(
            wv_sel.T.reshape(16, 128, 128).transpose(1, 0, 2)).astype(BF)
        # w_o columns for this core's heads; od-order hg=4h+g -> (odc=hg//2,
        # half=hg%2): wop[64*half+d, odc, j]
        wo_sel = w_o.reshape(D, 8, 4, 64)[:, 2 * p:2 * p + 2]   # [j, h, g, d]
        odT = wo_sel.transpose(1, 2, 3, 0).reshape(8, 64, D)    # [hg, d, j]
        wop = np.ascontiguousarray(
            odT.reshape(4, 2, 64, D).transpose(1, 2, 0, 3).reshape(128, 4, D)
        ).astype(BF)
        in_maps.append({
            "qt": qts[b], "kt": kts[b], "vt": vts[b],
            "wqp": wqp, "wkp": wkp, "wvp": wvp, "wop": wop, "eye": eye,
        })
    return in_maps


def kernel(Q, K, V, w_q, w_k, w_v, w_o, b_o):
    from concourse.bass_utils import run_bass_kernel_spmd
    nc = get_nc()
    in_maps = make_in_maps(Q, K, V, w_q, w_k, w_v, w_o)
    res = run_bass_kernel_spmd(nc, in_maps, core_ids=list(range(8)))
    out = np.zeros((2, N, D), dtype=np.float32)
    for c in range(8):
        b = c // 4
        out[b] += res.results[c]["outt"].astype(np.float32).T
    out += np.asarray(b_o, dtype=np.float32)[None, None, :]
    return out
